# revision 1
# baseline (speedup 1.0000x reference)
"""Trainium2 Bass kernel for nn_KC_Avg_Embedding (multi-hot averaged embedding).

Computes, for multi-hot indicator vectors x[b,s,:] over a vocabulary of 1024:
    out[b,s,:] = (x[b,s,:] @ E) / max(sum(x[b,s,:]), 1)

Strategy (data-parallel over 8 NeuronCores, batch-sharded):
  - Each core gets rows = (B/8)*S = 3200 rows of x [3200, 1024] fp32 plus the
    full embedding matrix E [1024, 128] fp32.
  - x is DMA'd in with an fp32->bf16 cast (SWDGE); x is 0/1 so bf16 is exact.
  - Each [128, 128] block of x is transposed on the TensorEngine (via
    identity-matmul transpose) so the vocab dim lands on partitions.
  - E is split on-chip into bf16 hi + lo parts (E = hi + lo to ~2^-17 rel) and
    extended with a ones column; 16 accumulating bf16 matmuls per row-tile
    produce [128 rows, 129] in PSUM = [x@E | row_count] with fp32 accumulation.
  - Epilogue: out = psum[:, :128] * (1 / max(psum[:, 128], 1)).
"""

import sys
from contextlib import ExitStack

import numpy as np

for _p in ("/opt/trn_rl_repo",):
    if _p not in sys.path:
        sys.path.insert(0, _p)

import concourse.bass as bass
import concourse.mybir as mybir
import concourse.tile as tile
from concourse.masks import make_identity

from concourse.vector_clock import ScopedClock


class _SplitDrainTC(tile.TileContext):
    """TileContext whose kernel-tail drain splits its semaphore waits across
    single-wait carrier nops — this walrus build enforces a small
    per-instruction sync-wait limit that the stock all-lane drain exceeds."""

    def _drain_and_barrier(self, tick_clock, wait_clock):
        drain_inst = self.nc.sync.drain()
        wait_clock.add_sem_waits(
            drain_inst.ins, ScopedClock({None: tick_clock.global_clock})
        )
        si = drain_inst.ins.sync_info
        if si is not None and si.on_wait is not None and len(si.on_wait) > 1:
            waits = list(si.on_wait)
            del si.on_wait[1:]
            for w in waits[1:]:
                nop = self.nc.sync.nop(nofuse=True, hint="drain_wait_split")
                nsi = nop.ins.sync_info
                if nsi is None:
                    nop.ins.sync_info = mybir.SyncInfo(on_update=[], on_wait=[w])
                else:
                    nsi.on_wait.append(w)
        self.nc.all_engine_barrier()
        assert self.sems is not None
        popped = self.nc._tile_sem_poison_stack.pop()
        assert popped is self._sem_poison
        self.nc.clear_and_free_semaphores(list(self.sems.allocated().values()))
        self.nc.all_engine_barrier()


B, S, V, D = 128, 200, 1024, 128
NCORES = 8
P = 128
PER_CORE_B = B // NCORES          # 16
ROWS = PER_CORE_B * S             # 3200 rows per core
NCH = V // P                      # 8 vocab chunks
NE = D + 1                        # 128 emb cols + 1 count col


def build_kernel(rows=ROWS, group=5):
    """Build the per-core Bass program. `rows` must be a multiple of 128*group."""
    rt = rows // P                 # row tiles
    assert rt % group == 0
    ng = rt // group               # DMA groups

    nc = bass.Bass()
    x = nc.declare_dram_parameter("x", [rows, V], mybir.dt.float32, isOutput=False)
    emb = nc.declare_dram_parameter("emb", [V, D], mybir.dt.float32, isOutput=False)
    y = nc.declare_dram_parameter("y", [rows, D], mybir.dt.float32, isOutput=True)

    bf16 = mybir.dt.bfloat16
    f32 = mybir.dt.float32

    with _SplitDrainTC(nc) as tc, ExitStack() as ctx:
        const = ctx.enter_context(tc.tile_pool(name="const", bufs=1))
        # one slot per group: avoids slot-reuse waits that push instructions
        # over walrus' one-sync-wait-per-instruction codegen limit
        xb_pool = ctx.enter_context(tc.tile_pool(name="xb", bufs=ng))
        xt_pool = ctx.enter_context(tc.tile_pool(name="xt", bufs=4))
        out_pool = ctx.enter_context(tc.tile_pool(name="out", bufs=ng))
        small = ctx.enter_context(tc.tile_pool(name="small", bufs=4))
        psum_t = ctx.enter_context(tc.tile_pool(name="psum_t", bufs=2, space="PSUM"))
        psum_o = ctx.enter_context(tc.tile_pool(name="psum_o", bufs=2, space="PSUM"))

        # identity for TensorE transposes
        ident = const.tile([P, P], bf16)
        make_identity(nc, ident)

        # E -> bf16 hi/lo split, chunked [p, chunk, d], plus ones/zeros count col
        e_f32 = const.tile([P, NCH, D], f32)
        nc.sync.dma_start(e_f32[:], emb.rearrange("(c p) d -> p c d", p=P))
        rhs_hi = const.tile([P, NCH, NE], bf16)
        rhs_lo = const.tile([P, NCH, NE], bf16)
        e_hi32 = const.tile([P, NCH, D], f32)
        nc.vector.tensor_copy(rhs_hi[:, :, 0:D], e_f32[:])      # round to bf16
        nc.vector.tensor_copy(e_hi32[:], rhs_hi[:, :, 0:D])     # widen back
        nc.vector.tensor_sub(rhs_lo[:, :, 0:D], e_f32[:], e_hi32[:])
        nc.vector.memset(rhs_hi[:, :, D:NE], 1.0)
        nc.vector.memset(rhs_lo[:, :, D:NE], 0.0)

        # row = (g*group + f)*128 + p
        xg = x.rearrange("(g f p) v -> g p f v", p=P, f=group)
        yg = y.rearrange("(g f p) d -> g p f d", p=P, f=group)

        for g in range(ng):
            xb = xb_pool.tile([P, group, V], bf16)
            nc.gpsimd.dma_start(xb[:], xg[g])  # fp32 -> bf16 cast during DMA
            out_sb = out_pool.tile([P, group, D], f32)
            for f in range(group):
                pt = psum_t.tile([P, NCH, P], bf16)
                for c in range(NCH):
                    nc.tensor.transpose(pt[:, c, :], xb[:, f, c * P:(c + 1) * P], ident)
                xt = xt_pool.tile([P, NCH, P], bf16)
                # PSUM -> SBUF copyback on DVE (ACT trips walrus'
                # per-instruction sync-wait limit in this dependency pattern)
                nc.vector.tensor_copy(xt[:, 0:4, :], pt[:, 0:4, :])
                nc.vector.tensor_copy(xt[:, 4:NCH, :], pt[:, 4:NCH, :])
                po = psum_o.tile([P, NE], f32)
                for c in range(NCH):
                    nc.tensor.matmul(po[:], xt[:, c, :], rhs_hi[:, c, :],
                                     start=(c == 0), stop=False)
                    nc.tensor.matmul(po[:], xt[:, c, :], rhs_lo[:, c, :],
                                     start=False, stop=(c == NCH - 1))
                r = small.tile([P, 1], f32)
                nc.vector.tensor_scalar_max(r[:], po[:, D:NE], 1.0)
                nc.vector.reciprocal(r[:], r[:])
                nc.vector.tensor_scalar_mul(out_sb[:, f, :], po[:, 0:D], r[:])
            nc.sync.dma_start(yg[g], out_sb[:])

    return nc


_cached_nc = None


def kernel(**inputs):
    global _cached_nc
    from concourse.bass_utils import run_bass_kernel_spmd

    x = np.asarray(inputs["batch_vectors"], dtype=np.float32).reshape(B, S, V)
    e = np.ascontiguousarray(np.asarray(inputs["embedding_matrix"], dtype=np.float32))

    if _cached_nc is None:
        _cached_nc = build_kernel()

    in_maps = []
    for i in range(NCORES):
        shard = np.ascontiguousarray(
            x[i * PER_CORE_B:(i + 1) * PER_CORE_B].reshape(ROWS, V)
        )
        in_maps.append({"x": shard, "emb": e})

    res = run_bass_kernel_spmd(_cached_nc, in_maps, core_ids=list(range(NCORES)))
    out = np.concatenate(
        [res.results[i]["y"].reshape(PER_CORE_B, S, D) for i in range(NCORES)],
        axis=0,
    )
    return out.astype(np.float32)



# revision 5
# speedup vs baseline: 1.7024x; 1.7024x over previous
"""Trainium2 Bass kernel for nn_KC_Avg_Embedding (multi-hot averaged embedding).

Computes, for multi-hot indicator vectors x[b,s,:] over a vocabulary of 1024:
    out[b,s,:] = (x[b,s,:] @ E) / max(sum(x[b,s,:]), 1)

Strategy (data-parallel over 8 NeuronCores, batch-sharded):
  - Each core gets rows = (B/8)*S = 3200 rows of x plus the full E [1024,128].
  - Host-side prep per core: x is 0/1 so it is encoded losslessly as fp8-e4m3
    bytes AND pre-transposed to [vocab, rows] tile layout -> the device does no
    transposes and reads 4x fewer HBM bytes than fp32.
  - E is hi/lo split into two fp8 parts (E = hi + lo to ~2^-8 rel) and extended
    with a ones/zeros column so the row count falls out of the same matmuls.
  - Device: per 128-row tile, 8 accumulating fp8 DoubleRow matmuls (K=256 each)
    produce [128 rows, 129] in PSUM = [x@E | count] with fp32 accumulation.
  - Epilogue on DVE: out = psum[:, :128] * (1 / max(psum[:, 128], 1)), written
    as bf16; host widens to fp32.
"""

import sys
from contextlib import ExitStack

import numpy as np
import ml_dtypes

for _p in ("/opt/trn_rl_repo",):
    if _p not in sys.path:
        sys.path.insert(0, _p)

import concourse.bass as bass
import concourse.mybir as mybir
import concourse.tile as tile

from concourse.vector_clock import ScopedClock


class _SplitDrainTC(tile.TileContext):
    """TileContext whose kernel-tail drain splits its semaphore waits across
    single-wait carrier nops — this walrus build enforces a small
    per-instruction sync-wait limit that the stock all-lane drain exceeds."""

    def _drain_and_barrier(self, tick_clock, wait_clock):
        drain_inst = self.nc.sync.drain()
        wait_clock.add_sem_waits(
            drain_inst.ins, ScopedClock({None: tick_clock.global_clock})
        )
        si = drain_inst.ins.sync_info
        if si is not None and si.on_wait is not None and len(si.on_wait) > 1:
            waits = list(si.on_wait)
            del si.on_wait[1:]
            for w in waits[1:]:
                nop = self.nc.sync.nop(nofuse=True, hint="drain_wait_split")
                nsi = nop.ins.sync_info
                if nsi is None:
                    nop.ins.sync_info = mybir.SyncInfo(on_update=[], on_wait=[w])
                else:
                    nsi.on_wait.append(w)
        self.nc.all_engine_barrier()
        assert self.sems is not None
        popped = self.nc._tile_sem_poison_stack.pop()
        assert popped is self._sem_poison
        self.nc.clear_and_free_semaphores(list(self.sems.allocated().values()))
        self.nc.all_engine_barrier()


def _split_matmul_waits(nc):
    """walrus allows only one sync wait on a Matmult. PSUM slot reuse puts two
    (bank-drain + consumer-done) on the accumulation-start matmuls; hoist all
    but one onto the directly preceding Ldweights — same in-order PE stream,
    so blocking there first is equivalent."""
    for b in nc.m.functions[0].blocks:
        prev_pe = None
        for i in b.instructions:
            if getattr(i, "engine", None) != mybir.EngineType.PE:
                continue
            si = i.sync_info
            if (
                type(i).__name__ == "InstMatmult"
                and si is not None
                and si.on_wait is not None
                and len(si.on_wait) > 1
            ):
                assert prev_pe is not None and type(prev_pe).__name__ == "InstLdweights"
                moved = list(si.on_wait)[:-1]
                del si.on_wait[:-1]
                psi = prev_pe.sync_info
                if psi is None:
                    prev_pe.sync_info = mybir.SyncInfo(on_update=[], on_wait=moved)
                else:
                    for w in moved:
                        psi.on_wait.append(w)
                nw = len(prev_pe.sync_info.on_wait)
                assert nw <= 1, f"ldweights {prev_pe.name} now has {nw} waits"
            prev_pe = i


B, S, V, D = 128, 200, 1024, 128
NCORES = 8
P = 128
PER_CORE_B = B // NCORES          # 16
ROWS = PER_CORE_B * S             # 3200 rows per core
NE = D + 1                        # 128 emb cols + 1 count col
KT = V // (2 * P)                 # 4 k-tiles of 256 (DoubleRow pairs)
G = 5                             # DMA groups per core
F = ROWS // (G * P)               # 5 row tiles per group
RG = F * P                        # 640 rows per group

F8 = ml_dtypes.float8_e4m3
BF16 = ml_dtypes.bfloat16


def build_kernel():
    nc = bass.Bass()
    f8 = mybir.dt.float8e4
    bf16 = mybir.dt.bfloat16
    f32 = mybir.dt.float32
    dr = mybir.MatmulPerfMode.DoubleRow

    # x: [g, p(v), kt, i, r'] fp8; E: [p(v), kt, hi/lo, i, col] fp8
    xd = nc.declare_dram_parameter("x", [G, P, KT, 2, RG], f8, isOutput=False)
    ed = nc.declare_dram_parameter("emb", [P, KT, 2, 2, NE], f8, isOutput=False)
    yd = nc.declare_dram_parameter("y", [G, P, F, D], bf16, isOutput=True)

    with _SplitDrainTC(nc) as tc, ExitStack() as ctx:
        const = ctx.enter_context(tc.tile_pool(name="const", bufs=1))
        # one slot per group: avoids slot-reuse waits that push instructions
        # over walrus' one-sync-wait-per-instruction codegen limit
        xb_pool = ctx.enter_context(tc.tile_pool(name="xb", bufs=G))
        out_pool = ctx.enter_context(tc.tile_pool(name="out", bufs=G))
        small = ctx.enter_context(tc.tile_pool(name="small", bufs=4))
        psum_o = ctx.enter_context(tc.tile_pool(name="psum_o", bufs=4, space="PSUM"))

        psum_gate = ctx.enter_context(tc.tile_pool(name="psum_g", bufs=1, space="PSUM"))

        e_sb = const.tile([P, KT, 2, 2, NE], f8)
        nc.sync.dma_start(e_sb[:], ed[:])

        # Load all x groups up front, alternating the two HWDGE queues so the
        # loads stream concurrently and are never stuck behind an output store.
        xbs = []
        for g in range(G):
            xb = xb_pool.tile([P, KT, 2, RG], f8)
            (nc.sync if g % 2 == 0 else nc.scalar).dma_start(xb[:], xd[g])
            xbs.append(xb)

        pg = psum_gate.tile([P, NE], f32)
        for g in range(G):
            xb = xbs[g]
            # Gate matmul: absorbs this group's DMA-complete wait on the
            # in-order PE so the real matmuls below carry at most one sync
            # wait each (walrus enforces a tiny per-instruction wait limit).
            nc.tensor.matmul(pg[:], xb[:, 0, :, 0:P], e_sb[:, 0, 0],
                             start=True, stop=True, perf_mode=dr)
            out_sb = out_pool.tile([P, F, D], bf16)
            for f in range(F):
                po = psum_o.tile([P, NE], f32)
                lo = f * P
                for kt in range(KT):
                    xs = xb[:, kt, :, lo:lo + P]        # [128, 2, 128]
                    nc.tensor.matmul(po[:], xs, e_sb[:, kt, 0],
                                     start=(kt == 0), stop=False, perf_mode=dr)
                    nc.tensor.matmul(po[:], xs, e_sb[:, kt, 1],
                                     start=False, stop=(kt == KT - 1), perf_mode=dr)
                r = small.tile([P, 1], f32)
                nc.vector.tensor_scalar_max(r[:], po[:, D:NE], 1.0)
                nc.vector.reciprocal(r[:], r[:])
                nc.vector.tensor_scalar_mul(out_sb[:, f, :], po[:, 0:D], r[:])
            nc.gpsimd.dma_start(yd[g], out_sb[:])

    _split_matmul_waits(nc)
    return nc


def _prep_x_core(x_core):
    """[3200, 1024] 0/1 fp32 -> [G, 128, KT, 2, RG] fp8 bytes (v-transposed)."""
    # v = (kt*128 + p)*2 + i ; row g*640 + r'
    a = (x_core != 0).astype(np.uint8).reshape(G, RG, KT, P, 2)
    a = np.ascontiguousarray(a.transpose(0, 3, 2, 4, 1)) * np.uint8(0x38)
    return a.view(F8)


def _prep_emb(e):
    """[1024, 128] fp32 -> [128, KT, 2, 2, 129] fp8 (hi/lo split + count col)."""
    hi8 = e.astype(F8)
    lo8 = (e - hi8.astype(np.float32)).astype(F8)
    ones = np.full((V, 1), 0x38, np.uint8)
    h = np.concatenate([hi8.view(np.uint8), ones], axis=1)            # [V, 129]
    l = np.concatenate([lo8.view(np.uint8), np.zeros((V, 1), np.uint8)], axis=1)
    eb = np.stack([h, l]).reshape(2, KT, P, 2, NE)                    # [h,kt,p,i,j]
    return np.ascontiguousarray(eb.transpose(2, 1, 0, 3, 4)).view(F8)


def make_in_maps(batch_vectors, embedding_matrix):
    x = np.asarray(batch_vectors, dtype=np.float32).reshape(B, S, V)
    e = np.asarray(embedding_matrix, dtype=np.float32).reshape(V, D)
    ed = _prep_emb(e)
    in_maps = []
    for i in range(NCORES):
        shard = x[i * PER_CORE_B:(i + 1) * PER_CORE_B].reshape(ROWS, V)
        in_maps.append({"x": _prep_x_core(shard), "emb": ed})
    return in_maps


def unshard_output(results):
    outs = []
    for i in range(NCORES):
        y = np.asarray(results[i]["y"])                   # [G, 128, F, D] bf16
        y = y.transpose(0, 2, 1, 3).reshape(PER_CORE_B, S, D)
        outs.append(y.astype(np.float32))
    return np.concatenate(outs, axis=0)


_cached_nc = None


def kernel(**inputs):
    global _cached_nc
    from concourse.bass_utils import run_bass_kernel_spmd

    if _cached_nc is None:
        _cached_nc = build_kernel()

    in_maps = make_in_maps(inputs["batch_vectors"], inputs["embedding_matrix"])
    res = run_bass_kernel_spmd(_cached_nc, in_maps, core_ids=list(range(NCORES)))
    return unshard_output(res.results)


# revision 12
# speedup vs baseline: 1.7815x; 1.0465x over previous
"""Trainium2 Bass kernel for nn_KC_Avg_Embedding (multi-hot averaged embedding).

Computes, for multi-hot indicator vectors x[b,s,:] over a vocabulary of 1024:
    out[b,s,:] = (x[b,s,:] @ E) / max(sum(x[b,s,:]), 1)

Strategy (data-parallel over 8 NeuronCores, batch-sharded):
  - Each core gets rows = (B/8)*S = 3200 rows of x plus the full E [1024,128].
  - Host-side prep per core: x is 0/1 so it is encoded losslessly as fp8-e4m3
    bytes AND pre-transposed to [vocab, rows] tile layout -> the device does no
    transposes and reads 4x fewer HBM bytes than fp32.
  - E is hi/lo split into two fp8 parts (E = hi + lo to ~2^-8 rel) and packed
    with a ones column into a single 257-wide moving operand per k-tile:
    [hi(128) | lo(128) | ones].
  - Device: per 128-row tile, 4 accumulating fp8 DoubleRow matmuls (K=256
    each) produce [128 rows, 257] = [x@E_hi | x@E_lo | count] in PSUM with
    fp32 accumulation. PSUM tiles hold 3 row tiles (bank-aligned 512-col
    slots) so the epilogue is batched: r=1/max(count,1), tmp=po*r (bf16),
    tmp_hi+=tmp_lo, DMA out.
  - Host widens the bf16 output to fp32.
"""

import sys
from contextlib import ExitStack

import numpy as np
import ml_dtypes

for _p in ("/opt/trn_rl_repo",):
    if _p not in sys.path:
        sys.path.insert(0, _p)

import concourse.bass as bass
import concourse.mybir as mybir
import concourse.tile as tile

from concourse.vector_clock import ScopedClock


class _SplitDrainTC(tile.TileContext):
    """TileContext whose kernel-tail drain splits its semaphore waits across
    single-wait carrier nops — this walrus build enforces a small
    per-instruction sync-wait limit that the stock all-lane drain exceeds."""

    def _drain_and_barrier(self, tick_clock, wait_clock):
        drain_inst = self.nc.sync.drain()
        wait_clock.add_sem_waits(
            drain_inst.ins, ScopedClock({None: tick_clock.global_clock})
        )
        si = drain_inst.ins.sync_info
        if si is not None and si.on_wait is not None and len(si.on_wait) > 1:
            waits = list(si.on_wait)
            del si.on_wait[1:]
            for w in waits[1:]:
                nop = self.nc.sync.nop(nofuse=True, hint="drain_wait_split")
                nsi = nop.ins.sync_info
                if nsi is None:
                    nop.ins.sync_info = mybir.SyncInfo(on_update=[], on_wait=[w])
                else:
                    nsi.on_wait.append(w)
        self.nc.all_engine_barrier()
        assert self.sems is not None
        popped = self.nc._tile_sem_poison_stack.pop()
        assert popped is self._sem_poison
        self.nc.clear_and_free_semaphores(list(self.sems.allocated().values()))
        self.nc.all_engine_barrier()


def _split_matmul_waits(nc):
    """walrus allows only one sync wait on a Matmult. PSUM slot reuse puts two
    (bank-drain + consumer-done) on the accumulation-start matmuls; hoist all
    but one onto the directly preceding Ldweights — same in-order PE stream,
    so blocking there first is equivalent."""
    for b in nc.m.functions[0].blocks:
        prev_pe = None
        for i in b.instructions:
            if getattr(i, "engine", None) != mybir.EngineType.PE:
                continue
            si = i.sync_info
            if (
                type(i).__name__ == "InstMatmult"
                and si is not None
                and si.on_wait is not None
                and len(si.on_wait) > 1
            ):
                assert prev_pe is not None and type(prev_pe).__name__ == "InstLdweights"
                moved = list(si.on_wait)[:-1]
                del si.on_wait[:-1]
                psi = prev_pe.sync_info
                if psi is None:
                    prev_pe.sync_info = mybir.SyncInfo(on_update=[], on_wait=moved)
                else:
                    for w in moved:
                        psi.on_wait.append(w)
                nw = len(prev_pe.sync_info.on_wait)
                assert nw <= 1, f"ldweights {prev_pe.name} now has {nw} waits"
            prev_pe = i


B, S, V, D = 128, 200, 1024, 128
NCORES = 8
P = 128
PER_CORE_B = B // NCORES          # 16
ROWS = PER_CORE_B * S             # 3200 rows per core
T = ROWS // P                     # 25 row tiles
KT = V // (2 * P)                 # 4 k-tiles of 256 (DoubleRow pairs)
G = 5                             # x DMA groups per core
RG = ROWS // G                    # 640 rows per group
NC = 2 * D + 1                    # 257 moving cols: hi(128) | lo(128) | ones
FP = 3                            # row tiles per PSUM group (bank-aligned 512)

F8 = ml_dtypes.float8_e4m3
BF16 = ml_dtypes.bfloat16


def build_kernel():
    nc = bass.Bass()
    f8 = mybir.dt.float8e4
    bf16 = mybir.dt.bfloat16
    f32 = mybir.dt.float32
    dr = mybir.MatmulPerfMode.DoubleRow
    add = mybir.AluOpType.add
    mult = mybir.AluOpType.mult

    # x: [g, p(v), kt, i, r'] fp8; E: [p(v), kt, i, col] fp8; y: [t, p(row), d]
    xd = nc.declare_dram_parameter("x", [G, P, KT, 2, RG], f8, isOutput=False)
    ed = nc.declare_dram_parameter("emb", [P, KT, 2, NC], f8, isOutput=False)
    yd = nc.declare_dram_parameter("y", [T, P, D], bf16, isOutput=True)
    yr = yd.rearrange("t p d -> p t d")

    with _SplitDrainTC(nc) as tc, ExitStack() as ctx:
        const = ctx.enter_context(tc.tile_pool(name="const", bufs=1))
        # one slot per group: avoids slot-reuse waits that push instructions
        # over walrus' one-sync-wait-per-instruction codegen limit
        xb_pool = ctx.enter_context(tc.tile_pool(name="xb", bufs=G))
        small = ctx.enter_context(tc.tile_pool(name="small", bufs=2))
        psum_o = ctx.enter_context(tc.tile_pool(name="psum_o", bufs=2, space="PSUM"))
        psum_gate = ctx.enter_context(tc.tile_pool(name="psum_g", bufs=1, space="PSUM"))

        e_sb = const.tile([P, KT, 2, NC], f8)
        nc.sync.dma_start(e_sb[:], ed[:])

        # Load all x groups up front, alternating the two HWDGE queues so the
        # loads stream concurrently and are never stuck behind an output store.
        xbs = []
        for g in range(G):
            xb = xb_pool.tile([P, KT, 2, RG], f8)
            (nc.sync if g % 2 == 0 else nc.scalar).dma_start(xb[:], xd[g])
            xbs.append(xb)

        # single write-once output staging tile; 3 bulk stores on the (by
        # then idle) HWDGE queues. Avoids per-store SWDGE semaphores landing
        # extra waits on the epilogue ops.
        out_sb = const.tile([P, T, NC - 1], bf16)
        y_cuts = (9, 18, T)

        pg = psum_gate.tile([P, NC], f32)
        t = 0
        while t < T:
            n = min(FP, T - t)                 # row tiles in this PSUM group
            po = psum_o.tile([P, FP, 512], f32)
            for j in range(n):
                g, lo = (t + j) * P // RG, (t + j) * P % RG
                xb = xbs[g]
                if lo == 0:
                    # Gate matmul: absorbs group g's DMA-complete wait on the
                    # in-order PE so the real matmuls below carry at most one
                    # sync wait each (walrus allows only one per Matmult).
                    nc.tensor.matmul(pg[:], xb[:, 0, :, 0:P], e_sb[:, 0],
                                     start=True, stop=True, perf_mode=dr)
                for kt in range(KT):
                    nc.tensor.matmul(po[:, j, 0:NC], xb[:, kt, :, lo:lo + P],
                                     e_sb[:, kt], start=(kt == 0),
                                     stop=(kt == KT - 1), perf_mode=dr)
            # Batched epilogue over the PSUM group (a vector op may read only
            # one PSUM input): r = 1/max(count,1); tmp = po*r; tmp_hi += lo.
            r5 = small.tile([P, FP, 1], f32)
            nc.vector.tensor_scalar_max(r5[:, 0:n], po[:, 0:n, NC - 1:NC], 1.0)
            nc.vector.reciprocal(r5[:, 0:n], r5[:, 0:n])
            nc.vector.tensor_tensor(out_sb[:, t:t + n], po[:, 0:n, 0:NC - 1],
                                    r5[:, 0:n].broadcast_to([P, n, NC - 1]),
                                    op=mult)
            nc.vector.tensor_tensor(out_sb[:, t:t + n, 0:D], out_sb[:, t:t + n, 0:D],
                                    out_sb[:, t:t + n, D:2 * D], op=add)
            t0 = t
            t += n
            for ci, c in enumerate(y_cuts):
                if t0 < c <= t:
                    lo_c = (y_cuts[ci - 1] if ci else 0)
                    eng = (nc.sync, nc.scalar, nc.gpsimd)[ci]
                    eng.dma_start(yr[:, lo_c:c, :], out_sb[:, lo_c:c, 0:D])

    _split_matmul_waits(nc)
    return nc


def _prep_x_core(x_core):
    """[3200, 1024] 0/1 fp32 -> [G, 128, KT, 2, RG] fp8 bytes (v-transposed)."""
    # v = (kt*128 + p)*2 + i ; row g*640 + r'
    a = (x_core != 0).astype(np.uint8).reshape(G, RG, KT, P, 2)
    a = np.ascontiguousarray(a.transpose(0, 3, 2, 4, 1)) * np.uint8(0x38)
    return a.view(F8)


def _prep_emb(e):
    """[1024, 128] fp32 -> [128, KT, 2, 257] fp8: [hi(128) | lo(128) | ones]."""
    hi8 = e.astype(F8)
    lo8 = (e - hi8.astype(np.float32)).astype(F8)
    c = np.empty((V, NC), np.uint8)
    c[:, 0:D] = hi8.view(np.uint8)
    c[:, D:2 * D] = lo8.view(np.uint8)
    c[:, NC - 1] = 0x38                                   # 1.0 (count column)
    c = c.reshape(KT, P, 2, NC)                           # [kt, p, i, j]
    return np.ascontiguousarray(c.transpose(1, 0, 2, 3)).view(F8)


def make_in_maps(batch_vectors, embedding_matrix):
    x = np.asarray(batch_vectors, dtype=np.float32).reshape(B, S, V)
    e = np.asarray(embedding_matrix, dtype=np.float32).reshape(V, D)
    ed = _prep_emb(e)
    in_maps = []
    for i in range(NCORES):
        shard = x[i * PER_CORE_B:(i + 1) * PER_CORE_B].reshape(ROWS, V)
        in_maps.append({"x": _prep_x_core(shard), "emb": ed})
    return in_maps


def unshard_output(results):
    outs = []
    for i in range(NCORES):
        y = np.asarray(results[i]["y"])                   # [T, 128, D] bf16
        outs.append(y.reshape(PER_CORE_B, S, D).astype(np.float32))
    return np.concatenate(outs, axis=0)


_cached_nc = None


def kernel(**inputs):
    global _cached_nc
    from concourse.bass_utils import run_bass_kernel_spmd

    if _cached_nc is None:
        _cached_nc = build_kernel()

    in_maps = make_in_maps(inputs["batch_vectors"], inputs["embedding_matrix"])
    res = run_bass_kernel_spmd(_cached_nc, in_maps, core_ids=list(range(NCORES)))
    return unshard_output(res.results)
